# revision 2
# baseline (speedup 1.0000x reference)
"""Trainium2 Bass kernel for nn_ExpEig: out = U diag(exp(L)) U^T where
(L, U) = eigh(x) -- jnp.linalg.eigh symmetrizes its input, so this equals
expm(A) with A = (x + x^T)/2, computed WITHOUT eigendecomposition via
scaling-and-squaring:

    As = A / 2^s ;  T = Taylor_8(As) (Paterson-Stockmeyer) ;  out = T^(2^s)

All intermediates are symmetric polynomials in A, so PE matmuls
(lhsT.T @ rhs) need no transposes: P.T @ Q = P @ Q.  The only transpose
(building A from x) runs on the PE against an identity.

Sharding: data-parallel over the batch dim, 4096/8 = 512 matrices/core.
"""
import math
import numpy as np
from contextlib import ExitStack

import concourse.bacc as bacc
import concourse.mybir as mybir
import concourse.tile as tile
from concourse.bass_utils import run_bass_kernel_spmd

dt = mybir.dt

N = 128              # matrix dim == partition count
B_FULL = 4096        # total batch
N_CORES = 8
B_CORE = B_FULL // N_CORES   # 512 matrices per core
QUAD = 4                     # matrices processed per group (one [128,512] tile)
N_QUADS = B_CORE // QUAD     # 128

M_TAYLOR = 8
S_SQUARE = 5
SCALE = 1.0 / (1 << S_SQUARE)

# consts layout (columns of one [128, 640] fp32 tensor)
C_IQUAD = 0        # [128,512] identity tiled x4
C_IDENT = 512      # [128,128] identity
C_TOTAL = 640


def _consts_np() -> np.ndarray:
    ident = np.eye(N, dtype=np.float32)
    return np.concatenate([np.tile(ident, (1, QUAD)), ident], axis=1)


def build_bass():
    c = [np.float32(1.0 / math.factorial(k)) for k in range(M_TAYLOR + 1)]

    nc = bacc.Bacc()
    x = nc.dram_tensor("x", [B_CORE, N, N], dt.float32, kind="ExternalInput")
    consts = nc.dram_tensor("consts", [N, C_TOTAL], dt.float32, kind="ExternalInput")
    out = nc.dram_tensor("out", [B_CORE, N, N], dt.float32, kind="ExternalOutput")

    with tile.TileContext(nc) as tc, ExitStack() as ctx:
        cpool = ctx.enter_context(tc.tile_pool(name="consts", bufs=1))
        sb = ctx.enter_context(tc.tile_pool(name="sb", bufs=2))
        chain = ctx.enter_context(tc.tile_pool(name="chain", bufs=3))
        ps = ctx.enter_context(tc.tile_pool(name="ps", bufs=2, space="PSUM"))
        prod = ctx.enter_context(tc.tile_pool(name="prod", bufs=4, space="PSUM"))

        ct = cpool.tile([N, C_TOTAL], dt.float32, tag="consts")
        nc.sync.dma_start(ct[:], consts[:])
        iquad = ct[:, C_IQUAD:C_IQUAD + 512]
        ident = ct[:, C_IDENT:C_IDENT + 128]

        js = [slice(j * N, (j + 1) * N) for j in range(QUAD)]

        for q in range(N_QUADS):
            # ---- load x quad: SBUF[p, m*128+c] = x[4q+m, p, c]
            xq = sb.tile([N, QUAD * N], dt.float32, tag="xq")
            src = x[QUAD * q:QUAD * (q + 1)].rearrange("m p c -> p m c")
            nc.sync.dma_start(xq[:].rearrange("p (m c) -> p m c", c=N), src)

            # ---- build As = (x + x^T)/2 * 2^-s
            xs = sb.tile([N, QUAD * N], dt.float32, tag="xs")
            nc.vector.tensor_scalar_mul(xs[:], xq[:], SCALE * 0.5)
            psS = ps.tile([N, QUAD * N], dt.float32, tag="psS")
            for j in range(QUAD):
                nc.tensor.transpose(psS[:, js[j]], xs[:, js[j]], ident)
            As = sb.tile([N, QUAD * N], dt.float32, tag="As")
            nc.vector.tensor_tensor(As[:], psS[:], xs[:], op=mybir.AluOpType.add)

            # ---- powers: A2 = As@As, A3 = A2@As, A4 = A2@A2
            def products(lhs, rhs, tag):
                p = prod.tile([N, QUAD * N], dt.float32, tag="prod")
                for j in range(QUAD):
                    nc.tensor.matmul(p[:, js[j]], lhs[:, js[j]], rhs[:, js[j]],
                                     start=True, stop=True)
                t = sb.tile([N, QUAD * N], dt.float32, tag=tag)
                nc.scalar.copy(t[:], p[:])
                return t, p

            A2, _ = products(As, As, "A2")
            A3, _ = products(A2, As, "A3")
            A4, pA4 = products(A2, A2, "A4")

            # ---- M1 = c8*A4 + c7*A3 + c6*A2 + c5*As + c4*I
            m1 = chain.tile([N, QUAD * N], dt.float32, tag="m1")
            nc.scalar.mul(m1[:], pA4[:], float(c[8]))
            for coef, X in ((c[7], A3), (c[6], A2), (c[5], As)):
                m1b = chain.tile([N, QUAD * N], dt.float32, tag="m1")
                nc.vector.scalar_tensor_tensor(
                    m1b[:], X[:], float(coef), m1[:],
                    op0=mybir.AluOpType.mult, op1=mybir.AluOpType.add)
                m1 = m1b
            m1b = chain.tile([N, QUAD * N], dt.float32, tag="m1")
            nc.vector.scalar_tensor_tensor(
                m1b[:], iquad, float(c[4]), m1[:],
                op0=mybir.AluOpType.mult, op1=mybir.AluOpType.add)
            m1 = m1b

            # ---- T = M1@A4 + c3*A3 + c2*A2 + c1*As + c0*I
            pM2 = prod.tile([N, QUAD * N], dt.float32, tag="prod")
            for j in range(QUAD):
                nc.tensor.matmul(pM2[:, js[j]], m1[:, js[j]], A4[:, js[j]],
                                 start=True, stop=True)
            T = chain.tile([N, QUAD * N], dt.float32, tag="t")
            nc.vector.scalar_tensor_tensor(
                T[:], A3[:], float(c[3]), pM2[:],
                op0=mybir.AluOpType.mult, op1=mybir.AluOpType.add)
            for coef, X in ((c[2], A2), (c[1], As)):
                Tb = chain.tile([N, QUAD * N], dt.float32, tag="t")
                nc.vector.scalar_tensor_tensor(
                    Tb[:], X[:], float(coef), T[:],
                    op0=mybir.AluOpType.mult, op1=mybir.AluOpType.add)
                T = Tb
            Tb = chain.tile([N, QUAD * N], dt.float32, tag="t")
            nc.vector.scalar_tensor_tensor(
                Tb[:], iquad, float(c[0]), T[:],
                op0=mybir.AluOpType.mult, op1=mybir.AluOpType.add)
            T = Tb

            # ---- s squarings
            for k in range(S_SQUARE):
                pT = prod.tile([N, QUAD * N], dt.float32, tag="prod")
                for j in range(QUAD):
                    nc.tensor.matmul(pT[:, js[j]], T[:, js[j]], T[:, js[j]],
                                     start=True, stop=True)
                Tb = chain.tile([N, QUAD * N], dt.float32, tag="t")
                nc.scalar.copy(Tb[:], pT[:])
                T = Tb

            # ---- store
            dst = out[QUAD * q:QUAD * (q + 1)].rearrange("m p c -> p m c")
            nc.sync.dma_start(dst, T[:].rearrange("p (m c) -> p m c", c=N))

    nc.compile()
    return nc


_NC_CACHE = None


def kernel(x: np.ndarray) -> np.ndarray:
    global _NC_CACHE
    assert x.shape == (B_FULL, N, N) and x.dtype == np.float32, (x.shape, x.dtype)
    if _NC_CACHE is None:
        _NC_CACHE = build_bass()
    nc = _NC_CACHE

    consts = _consts_np()
    in_maps = [
        {"x": np.ascontiguousarray(x[i * B_CORE:(i + 1) * B_CORE]), "consts": consts}
        for i in range(N_CORES)
    ]
    res = run_bass_kernel_spmd(nc, in_maps, list(range(N_CORES))).results
    return np.concatenate([r["out"] for r in res], axis=0)


# revision 6
# speedup vs baseline: 1.2202x; 1.2202x over previous
"""Trainium2 Bass kernel for nn_ExpEig: out = U diag(exp(L)) U^T where
(L, U) = eigh(x) -- jnp.linalg.eigh symmetrizes its input, so this equals
expm(A) with A = (x + x^T)/2, computed WITHOUT eigendecomposition via
scaling-and-squaring:

    As = A / 2^s ;  T = Taylor_8(As) (Paterson-Stockmeyer) ;  out = T^(2^s)

All intermediates are symmetric polynomials in A, so PE matmuls
(lhsT.T @ rhs) need no transposes: P.T @ Q = P @ Q.  The only transpose
(building A from x) runs on the PE against an identity.

Sharding: data-parallel over the batch dim, 4096/8 = 512 matrices/core.
"""
import math
import numpy as np
from contextlib import ExitStack

import concourse.bacc as bacc
import concourse.mybir as mybir
import concourse.tile as tile
from concourse.bass_utils import run_bass_kernel_spmd

dt = mybir.dt

N = 128              # matrix dim == partition count
B_FULL = 4096        # total batch
N_CORES = 8
B_CORE = B_FULL // N_CORES   # 512 matrices per core
QUAD = 4                     # matrices processed per group (one [128,512] tile)
N_QUADS = B_CORE // QUAD     # 128

M_TAYLOR = 8
S_SQUARE = 5
SCALE = 1.0 / (1 << S_SQUARE)

# consts layout (columns of one [128, 640] fp32 tensor)
C_IQUAD = 0        # [128,512] identity tiled x4
C_IDENT = 512      # [128,128] identity
C_TOTAL = 640


def _consts_np() -> np.ndarray:
    ident = np.eye(N, dtype=np.float32)
    return np.concatenate([np.tile(ident, (1, QUAD)), ident], axis=1)


def build_bass():
    c = [np.float32(1.0 / math.factorial(k)) for k in range(M_TAYLOR + 1)]

    nc = bacc.Bacc()
    x = nc.dram_tensor("x", [B_CORE, N, N], dt.float32, kind="ExternalInput")
    consts = nc.dram_tensor("consts", [N, C_TOTAL], dt.float32, kind="ExternalInput")
    out = nc.dram_tensor("out", [B_CORE, N, N], dt.float32, kind="ExternalOutput")

    with tile.TileContext(nc) as tc, ExitStack() as ctx:
        cpool = ctx.enter_context(tc.tile_pool(name="consts", bufs=1))
        sb = ctx.enter_context(tc.tile_pool(name="sb", bufs=3))
        chain = ctx.enter_context(tc.tile_pool(name="chain", bufs=6))
        ps = ctx.enter_context(tc.tile_pool(name="ps", bufs=2, space="PSUM"))
        prod = ctx.enter_context(tc.tile_pool(name="prod", bufs=6, space="PSUM"))

        ct = cpool.tile([N, C_TOTAL], dt.float32, tag="consts")
        nc.sync.dma_start(ct[:], consts[:])
        iquad = ct[:, C_IQUAD:C_IQUAD + 512]
        ident = ct[:, C_IDENT:C_IDENT + 128]

        js = [slice(j * N, (j + 1) * N) for j in range(QUAD)]

        GRP = 4   # quads per DMA instruction (1 MiB transfers)
        xg_tiles = {}
        og_tiles = {}

        def stage_load(q):
            """DMA group load (every GRP quads) + build As = (x+x^T)/2 * 2^-s."""
            g = q // GRP
            if q % GRP == 0:
                xg = sb.tile([N, GRP * QUAD * N], dt.float32, tag="xg")
                srcap = x[GRP * QUAD * g:GRP * QUAD * (g + 1)].rearrange(
                    "m p c -> p m c")
                nc.sync.dma_start(xg[:].rearrange("p (m c) -> p m c", c=N), srcap)
                xg_tiles[g] = xg
            xq = xg_tiles[g][:, (q % GRP) * QUAD * N:((q % GRP) + 1) * QUAD * N]

            xs = sb.tile([N, QUAD * N], dt.float32, tag="xs")
            nc.vector.tensor_scalar_mul(xs[:], xq[:], SCALE * 0.5)
            psS = ps.tile([N, QUAD * N], dt.float32, tag="psS")
            for j in range(QUAD):
                nc.tensor.transpose(psS[:, js[j]], xs[:, js[j]], ident)
            As = sb.tile([N, QUAD * N], dt.float32, tag="As")
            nc.vector.tensor_tensor(As[:], psS[:], xs[:], op=mybir.AluOpType.add)
            return As

        def emit_products(lhs, rhs):
            p = prod.tile([N, QUAD * N], dt.float32, tag="prod")
            for j in range(QUAD):
                nc.tensor.matmul(p[:, js[j]], lhs[:, js[j]], rhs[:, js[j]],
                                 start=True, stop=True)
            return p

        def copy_out(p, tag):
            t = sb.tile([N, QUAD * N], dt.float32, tag=tag)
            nc.scalar.copy(t[:], p[:])
            return t

        def stage_powers(As):
            """A2/A3/A4 + the M1 coefficient chain."""
            pA2 = emit_products(As, As)
            A2 = copy_out(pA2, "A2")
            pA3 = emit_products(A2, As)
            A3 = copy_out(pA3, "A3")
            pA4 = emit_products(A2, A2)
            A4 = copy_out(pA4, "A4")

            m1 = chain.tile([N, QUAD * N], dt.float32, tag="m1")
            nc.scalar.mul(m1[:], pA4[:], float(c[8]))
            for coef, X in ((c[7], A3), (c[6], A2), (c[5], As)):
                m1b = chain.tile([N, QUAD * N], dt.float32, tag="m1")
                nc.vector.scalar_tensor_tensor(
                    m1b[:], X[:], float(coef), m1[:],
                    op0=mybir.AluOpType.mult, op1=mybir.AluOpType.add)
                m1 = m1b
            m1b = chain.tile([N, QUAD * N], dt.float32, tag="m1")
            nc.vector.scalar_tensor_tensor(
                m1b[:], iquad, float(c[4]), m1[:],
                op0=mybir.AluOpType.mult, op1=mybir.AluOpType.add)
            return m1b, As, A2, A3, A4

        def stage_m2(st):
            m1, As, A2, A3, A4 = st
            pM2 = emit_products(m1, A4)
            return pM2, As, A2, A3

        def stage_tchain(st):
            pM2, As, A2, A3 = st
            T = chain.tile([N, QUAD * N], dt.float32, tag="t")
            nc.vector.scalar_tensor_tensor(
                T[:], A3[:], float(c[3]), pM2[:],
                op0=mybir.AluOpType.mult, op1=mybir.AluOpType.add)
            for coef, X in ((c[2], A2), (c[1], As)):
                Tb = chain.tile([N, QUAD * N], dt.float32, tag="t")
                nc.vector.scalar_tensor_tensor(
                    Tb[:], X[:], float(coef), T[:],
                    op0=mybir.AluOpType.mult, op1=mybir.AluOpType.add)
                T = Tb
            Tb = chain.tile([N, QUAD * N], dt.float32, tag="t")
            nc.vector.scalar_tensor_tensor(
                Tb[:], iquad, float(c[0]), T[:],
                op0=mybir.AluOpType.mult, op1=mybir.AluOpType.add)
            return Tb

        def stage_square(q, T, k):
            pT = emit_products(T, T)
            if k < S_SQUARE - 1:
                return copy_out(pT, "t2")
            # final: stage into the grouped out tile
            g = q // GRP
            if q % GRP == 0:
                og_t = sb.tile([N, GRP * QUAD * N], dt.float32, tag="og")
                og_tiles[g] = og_t
            stage = og_tiles[g]
            nc.scalar.copy(
                stage[:, (q % GRP) * QUAD * N:((q % GRP) + 1) * QUAD * N], pT[:])
            if q % GRP == GRP - 1:
                dst = out[GRP * QUAD * g:GRP * QUAD * (g + 1)].rearrange(
                    "m p c -> p m c")
                nc.sync.dma_start(
                    dst, og_tiles[g][:].rearrange("p (m c) -> p m c", c=N))
            return None

        # rolling software pipeline, 2 pairs in flight:
        #   iter i emits: loads+powers+M2+Tchain of pair i, squarings of pair i-1
        # interleaved so PE fills pair i-1's chain-waits with pair i's matmuls.
        prev_pair = None
        prev_T = None

        def emit_squares(pair, T01, ks):
            for k in ks:
                for i in range(2):
                    T01[i] = stage_square(pair[i], T01[i], k)
            return T01

        for q0 in range(0, N_QUADS, 2):
            pair = [q0, q0 + 1]
            As01 = [stage_load(q) for q in pair]
            st_a = stage_powers(As01[0])
            if prev_pair is not None:
                prev_T = emit_squares(prev_pair, prev_T, [0])
            st_b = stage_powers(As01[1])
            if prev_pair is not None:
                prev_T = emit_squares(prev_pair, prev_T, range(1, S_SQUARE))
            st01 = [stage_m2(st_a), stage_m2(st_b)]
            T01 = [stage_tchain(st01[0]), stage_tchain(st01[1])]
            prev_pair, prev_T = pair, T01

        emit_squares(prev_pair, prev_T, range(S_SQUARE))

    nc.compile()
    return nc


_NC_CACHE = None


def kernel(x: np.ndarray) -> np.ndarray:
    global _NC_CACHE
    assert x.shape == (B_FULL, N, N) and x.dtype == np.float32, (x.shape, x.dtype)
    if _NC_CACHE is None:
        _NC_CACHE = build_bass()
    nc = _NC_CACHE

    consts = _consts_np()
    in_maps = [
        {"x": np.ascontiguousarray(x[i * B_CORE:(i + 1) * B_CORE]), "consts": consts}
        for i in range(N_CORES)
    ]
    res = run_bass_kernel_spmd(nc, in_maps, list(range(N_CORES))).results
    return np.concatenate([r["out"] for r in res], axis=0)


# revision 18
# speedup vs baseline: 95.7846x; 78.5020x over previous
"""Trainium2 Bass kernel for nn_ExpEig: out = U diag(exp(L)) U^T where
(L, U) = eigh(x) -- jnp.linalg.eigh symmetrizes its input, so this equals
expm(A) with A = (x + x^T)/2, computed WITHOUT eigendecomposition via
scaling-and-squaring:

    As = A / 2^s ;  T = Taylor_12(As) (Paterson-Stockmeyer) ;  out = T^(2^s)

All intermediates are symmetric polynomials in A, so PE matmuls
(lhsT.T @ rhs) need no transposes: P.T @ Q = P @ Q.  The only transpose
(building A from x) runs on the PE against an identity.

Sharding: data-parallel over the batch dim, 4096/8 = 512 matrices/core.
"""
import math
import numpy as np
from contextlib import ExitStack

import concourse.bacc as bacc
import concourse.mybir as mybir
import concourse.tile as tile
from concourse.bass_utils import run_bass_kernel_spmd

dt = mybir.dt

N = 128              # matrix dim == partition count
B_FULL = 4096        # total batch
N_CORES = 8
B_CORE = B_FULL // N_CORES   # 512 matrices per core
QUAD = 4                     # matrices processed per group (one [128,512] tile)
N_QUADS = B_CORE // QUAD     # 128

M_TAYLOR = 10
S_SQUARE = 4
SCALE = 1.0 / (1 << S_SQUARE)

# consts layout (columns of one [128, 640] fp32 tensor)
C_IQUAD = 0        # [128,512] identity tiled x4
C_IDENT = 512      # [128,128] identity
C_TOTAL = 640


def _consts_np() -> np.ndarray:
    ident = np.eye(N, dtype=np.float32)
    return np.concatenate([np.tile(ident, (1, QUAD)), ident], axis=1)


def build_bass(repeat: int = 1):
    c = [np.float32(1.0 / math.factorial(k)) for k in range(M_TAYLOR + 1)]

    nc = bacc.Bacc()
    x = nc.dram_tensor("x", [B_CORE, N, N], dt.float32, kind="ExternalInput")
    consts = nc.dram_tensor("consts", [N, C_TOTAL], dt.float32, kind="ExternalInput")
    out = nc.dram_tensor("out", [B_CORE, N, N], dt.float32, kind="ExternalOutput")

    with tile.TileContext(nc) as tc, ExitStack() as ctx:
        cpool = ctx.enter_context(tc.tile_pool(name="consts", bufs=1))
        sb = ctx.enter_context(tc.tile_pool(name="sb", bufs=4))
        chain = ctx.enter_context(tc.tile_pool(name="chain", bufs=10))
        ps = ctx.enter_context(tc.tile_pool(name="ps", bufs=2, space="PSUM"))
        prod = ctx.enter_context(tc.tile_pool(name="prod", bufs=6, space="PSUM"))

        ct = cpool.tile([N, C_TOTAL], dt.float32, tag="consts")
        nc.sync.dma_start(ct[:], consts[:])
        iquad = ct[:, C_IQUAD:C_IQUAD + 512]
        ident = ct[:, C_IDENT:C_IDENT + 128]

        js = [slice(j * N, (j + 1) * N) for j in range(QUAD)]

        GRP = 4   # quads per DMA instruction (1 MiB transfers)
        xg_tiles = {}
        og_tiles = {}

        def stage_load(q):
            """DMA group load (every GRP quads) + build As = (x+x^T)/2 * 2^-s."""
            g = (rep, q // GRP)
            if q % GRP == 0:
                xg = sb.tile([N, GRP * QUAD * N], dt.float32, tag="xg")
                gq = q // GRP
                srcap = x[GRP * QUAD * gq:GRP * QUAD * (gq + 1)].rearrange(
                    "m p c -> p m c")
                nc.sync.dma_start(xg[:].rearrange("p (m c) -> p m c", c=N), srcap)
                xg_tiles[g] = xg
            xq = xg_tiles[g][:, (q % GRP) * QUAD * N:((q % GRP) + 1) * QUAD * N]

            xs = sb.tile([N, QUAD * N], dt.float32, tag="xs")
            nc.scalar.mul(xs[:], xq[:], SCALE * 0.5)
            psS = ps.tile([N, QUAD * N], dt.float32, tag="psS")
            for j in range(QUAD):
                nc.tensor.transpose(psS[:, js[j]], xs[:, js[j]], ident)
            As = sb.tile([N, QUAD * N], dt.float32, tag="As")
            nc.vector.tensor_tensor(As[:], psS[:], xs[:], op=mybir.AluOpType.add)
            return As

        def emit_products(lhs, rhs):
            p = prod.tile([N, QUAD * N], dt.float32, tag="prod")
            for j in range(QUAD):
                nc.tensor.matmul(p[:, js[j]], lhs[:, js[j]], rhs[:, js[j]],
                                 start=True, stop=True)
            return p

        def copy_out(p, tag):
            t = sb.tile([N, QUAD * N], dt.float32, tag=tag)
            nc.scalar.copy(t[:], p[:])
            return t

        def coeff_chain(seed, k0, tag):
            """seed + c[k0+3]*A3 + c[k0+2]*A2 + c[k0+1]*As + c[k0]*I (DVE)."""
            cur = seed["tile"]
            As, A2, A3 = seed["basis"]
            for coef, X in ((c[k0 + 3], A3), (c[k0 + 2], A2), (c[k0 + 1], As),
                            (c[k0], iquad)):
                nxt = chain.tile([N, QUAD * N], dt.float32, tag=tag)
                nc.vector.scalar_tensor_tensor(
                    nxt[:], X[:] if hasattr(X, "shape") else X, float(coef), cur[:],
                    op0=mybir.AluOpType.mult, op1=mybir.AluOpType.add)
                cur = nxt
            return cur

        def stage_powers(As):
            """A2/A3/A4 + top chain M1 = c10*A2 + c9*As + c8*I (seed rides pA2)."""
            pA2 = emit_products(As, As)
            A2 = copy_out(pA2, "A2")
            m1 = chain.tile([N, QUAD * N], dt.float32, tag="m1")
            nc.scalar.mul(m1[:], pA2[:], float(c[10]))
            pA3 = emit_products(A2, As)
            A3 = copy_out(pA3, "A3")
            pA4 = emit_products(A2, A2)
            A4 = copy_out(pA4, "A4")
            for coef, X in ((c[9], As), (c[8], iquad)):
                nxt = chain.tile([N, QUAD * N], dt.float32, tag="m1")
                nc.vector.scalar_tensor_tensor(
                    nxt[:], X[:] if hasattr(X, "shape") else X, float(coef), m1[:],
                    op0=mybir.AluOpType.mult, op1=mybir.AluOpType.add)
                m1 = nxt
            return m1, As, A2, A3, A4

        def stage_m2(st):
            """M2 = M1 @ A4  (chain C1 appended separately)."""
            m1, As, A2, A3, A4 = st
            pM2 = emit_products(m1, A4)
            return pM2, As, A2, A3, A4

        def stage_c1chain(st):
            pM2, As, A2, A3, A4 = st
            m2 = coeff_chain({"tile": pM2, "basis": (As, A2, A3)}, 4, "m2")
            return m2, As, A2, A3, A4

        def stage_m3(st):
            m2, As, A2, A3, A4 = st
            pM3 = emit_products(m2, A4)
            return pM3, As, A2, A3

        def stage_tchain(st):
            pM3, As, A2, A3 = st
            return coeff_chain({"tile": pM3, "basis": (As, A2, A3)}, 0, "t")

        def stage_square(q, T, k):
            pT = emit_products(T, T)
            if k < S_SQUARE - 1:
                return copy_out(pT, "t2")
            # final: stage into the grouped out tile
            g = q // GRP
            if q % GRP == 0:
                og_t = sb.tile([N, GRP * QUAD * N], dt.float32, tag="og")
                og_tiles[g] = og_t
            stage = og_tiles[g]
            nc.scalar.copy(
                stage[:, (q % GRP) * QUAD * N:((q % GRP) + 1) * QUAD * N], pT[:])
            if q % GRP == GRP - 1:
                gq = q // GRP
                dst = out[GRP * QUAD * gq:GRP * QUAD * (gq + 1)].rearrange(
                    "m p c -> p m c")
                nc.sync.dma_start(
                    dst, og_tiles[g][:].rearrange("p (m c) -> p m c", c=N))
            return None

        # rolling software pipeline, 2 pairs in flight:
        #   iter i emits: loads+powers+M2+Tchain of pair i, squarings of pair i-1
        # interleaved so PE fills pair i-1's chain-waits with pair i's matmuls.
        prev_pair = None
        prev_T = None

        def emit_squares(pair, T01, ks):
            for k in ks:
                for i in range(2):
                    T01[i] = stage_square(pair[i], T01[i], k)
            return T01

        for rep in range(repeat):
          for q0 in range(0, N_QUADS, 2):
            pair = [q0, q0 + 1]
            As01 = [stage_load(q) for q in pair]
            st_a = stage_powers(As01[0])
            if prev_pair is not None:
                prev_T = emit_squares(prev_pair, prev_T, [0])
            st_b = stage_powers(As01[1])
            if prev_pair is not None:
                prev_T = emit_squares(prev_pair, prev_T, [1])
            st01 = [stage_m2(st_a), stage_m2(st_b)]
            if prev_pair is not None:
                prev_T = emit_squares(prev_pair, prev_T, [2])
            st01 = [stage_c1chain(st01[0]), stage_c1chain(st01[1])]
            if prev_pair is not None:
                prev_T = emit_squares(prev_pair, prev_T, [3])
            st01 = [stage_m3(st01[0]), stage_m3(st01[1])]
            T01 = [stage_tchain(st01[0]), stage_tchain(st01[1])]
            prev_pair, prev_T = pair, T01

        emit_squares(prev_pair, prev_T, range(S_SQUARE))

    nc.compile()
    return nc


_NC_CACHE = None


def kernel(x) -> np.ndarray:
    global _NC_CACHE
    x = np.asarray(x, dtype=np.float32)
    assert x.shape == (B_FULL, N, N), x.shape
    if _NC_CACHE is None:
        _NC_CACHE = build_bass()
    nc = _NC_CACHE

    consts = _consts_np()
    in_maps = [
        {"x": np.ascontiguousarray(x[i * B_CORE:(i + 1) * B_CORE]), "consts": consts}
        for i in range(N_CORES)
    ]
    res = run_bass_kernel_spmd(nc, in_maps, list(range(N_CORES))).results
    return np.concatenate([r["out"] for r in res], axis=0)


# revision 23
# speedup vs baseline: 102.8779x; 1.0741x over previous
"""Trainium2 Bass kernel for nn_ExpEig: out = U diag(exp(L)) U^T where
(L, U) = eigh(x) -- jnp.linalg.eigh symmetrizes its input, so this equals
expm(A) with A = (x + x^T)/2, computed WITHOUT eigendecomposition via
scaling-and-squaring:

    As = A / 2^s ;  T = Taylor_10(As) (Paterson-Stockmeyer, s=4) ;  out = T^(2^s)

All intermediates are symmetric polynomials in A, so PE matmuls
(lhsT.T @ rhs) need no transposes: P.T @ Q = P @ Q.  The only transpose
(building A from x) runs on the PE against an identity.

Sharding: data-parallel over the batch dim, 4096/8 = 512 matrices/core.
"""
import math
import numpy as np
from contextlib import ExitStack

import concourse.bacc as bacc
import concourse.mybir as mybir
import concourse.tile as tile
from concourse.bass_utils import run_bass_kernel_spmd

dt = mybir.dt

N = 128              # matrix dim == partition count
B_FULL = 4096        # total batch
N_CORES = 8
B_CORE = B_FULL // N_CORES   # 512 matrices per core
QUAD = 4                     # matrices processed per group (one [128,512] tile)
N_QUADS = B_CORE // QUAD     # 128

M_TAYLOR = 10
S_SQUARE = 4
SCALE = 1.0 / (1 << S_SQUARE)

# consts layout (columns of one [128, 1664] fp32 tensor)
C_IQUAD = 0        # [128,512] identity tiled x4
C_IQUAD_C4 = 512   # [128,512] (1/4!)*identity tiled x4
C_IQUAD_C8 = 1024  # [128,512] (1/8!)*identity tiled x4
C_IDENT = 1536     # [128,128] identity
C_TOTAL = 1664


def _consts_np() -> np.ndarray:
    ident = np.eye(N, dtype=np.float32)
    iq = np.tile(ident, (1, QUAD))
    return np.concatenate([iq, iq / math.factorial(4), iq / math.factorial(8),
                           ident], axis=1)


def build_bass(repeat: int = 1):
    c = [np.float32(1.0 / math.factorial(k)) for k in range(M_TAYLOR + 1)]

    nc = bacc.Bacc()
    x = nc.dram_tensor("x", [B_CORE, N, N], dt.float32, kind="ExternalInput")
    consts = nc.dram_tensor("consts", [N, C_TOTAL], dt.float32, kind="ExternalInput")
    out = nc.dram_tensor("out", [B_CORE, N, N], dt.float32, kind="ExternalOutput")

    with tile.TileContext(nc) as tc, ExitStack() as ctx:
        cpool = ctx.enter_context(tc.tile_pool(name="consts", bufs=1))
        sb = ctx.enter_context(tc.tile_pool(name="sb", bufs=4))
        chain = ctx.enter_context(tc.tile_pool(name="chain", bufs=4))
        ps = ctx.enter_context(tc.tile_pool(name="ps", bufs=2, space="PSUM"))
        prod = ctx.enter_context(tc.tile_pool(name="prod", bufs=6, space="PSUM"))

        ct = cpool.tile([N, C_TOTAL], dt.float32, tag="consts")
        nc.sync.dma_start(ct[:], consts[:])
        iquad = ct[:, C_IQUAD:C_IQUAD + 512]
        iquad_c4 = ct[:, C_IQUAD_C4:C_IQUAD_C4 + 512]
        iquad_c8 = ct[:, C_IQUAD_C8:C_IQUAD_C8 + 512]
        ident = ct[:, C_IDENT:C_IDENT + 128]

        js = [slice(j * N, (j + 1) * N) for j in range(QUAD)]

        GRP = 4   # quads per DMA instruction (1 MiB transfers)
        xg_tiles = {}
        og_tiles = {}

        def stage_load(q):
            """DMA group load (every GRP quads) + build As = (x+x^T)/2 * 2^-s."""
            g = (rep, q // GRP)
            if q % GRP == 0:
                xg = sb.tile([N, GRP * QUAD * N], dt.float32, tag="xg")
                gq = q // GRP
                srcap = x[GRP * QUAD * gq:GRP * QUAD * (gq + 1)].rearrange(
                    "m p c -> p m c")
                nc.sync.dma_start(xg[:].rearrange("p (m c) -> p m c", c=N), srcap)
                xg_tiles[g] = xg
            xq = xg_tiles[g][:, (q % GRP) * QUAD * N:((q % GRP) + 1) * QUAD * N]

            xs = sb.tile([N, QUAD * N], dt.float32, tag="xs")
            nc.scalar.mul(xs[:], xq[:], SCALE * 0.5)
            psS = ps.tile([N, QUAD * N], dt.float32, tag="psS")
            for j in range(QUAD):
                nc.tensor.transpose(psS[:, js[j]], xs[:, js[j]], ident)
            As = sb.tile([N, QUAD * N], dt.float32, tag="As")
            nc.vector.tensor_tensor(As[:], psS[:], xs[:], op=mybir.AluOpType.add)
            return As

        def emit_products(lhs, rhs):
            p = prod.tile([N, QUAD * N], dt.float32, tag="prod")
            for j in range(QUAD):
                nc.tensor.matmul(p[:, js[j]], lhs[:, js[j]], rhs[:, js[j]],
                                 start=True, stop=True)
            return p

        def copy_out(p, tag):
            t = sb.tile([N, QUAD * N], dt.float32, tag=tag)
            nc.scalar.copy(t[:], p[:])
            return t

        def stt(out_tag, X, coef, Y):
            """new_tile = coef*X + Y on DVE."""
            nxt = chain.tile([N, QUAD * N], dt.float32, tag=out_tag)
            nc.vector.scalar_tensor_tensor(
                nxt[:], X[:], float(coef), Y[:],
                op0=mybir.AluOpType.mult, op1=mybir.AluOpType.add)
            return nxt

        def coeff_tree(pM, k0, W, basis, tag):
            """pM + c[k0+3]*A3 + (c[k0+2]*A2 + W), W precomputed (depth 2)."""
            As, A2, A3 = basis
            t1 = stt(tag, A3, c[k0 + 3], pM)
            t2 = stt(tag + "w", A2, c[k0 + 2], W)
            return stt(tag, t1, 1.0, t2)

        def stage_powers(As):
            """A2/A3/A4 + top chain M1 = c10*A2 + (c9*As + c8*I); the As/I
            partial sums W* for all three groups are emitted here, early."""
            w_top = stt("wt", As, c[9], iquad_c8)     # c9*As + c8*I
            w1 = stt("w1", As, c[5], iquad_c4)        # c5*As + c4*I
            w0 = stt("w0", As, c[1], iquad)           # c1*As + c0*I  (c0=c1=1)
            pA2 = emit_products(As, As)
            A2 = copy_out(pA2, "A2")
            m1s = chain.tile([N, QUAD * N], dt.float32, tag="m1")
            nc.scalar.mul(m1s[:], pA2[:], float(c[10]))
            pA3 = emit_products(A2, As)
            A3 = copy_out(pA3, "A3")
            pA4 = emit_products(A2, A2)
            A4 = copy_out(pA4, "A4")
            m1 = stt("m1", w_top, 1.0, m1s)
            return m1, As, A2, A3, A4, w1, w0

        def stage_m2(st):
            """M2 = M1 @ A4  (tree chain C1 appended separately)."""
            m1, As, A2, A3, A4, w1, w0 = st
            pM2 = emit_products(m1, A4)
            return pM2, As, A2, A3, A4, w1, w0

        def stage_c1chain(st):
            pM2, As, A2, A3, A4, w1, w0 = st
            m2 = coeff_tree(pM2, 4, w1, (As, A2, A3), "m2")
            return m2, As, A2, A3, A4, w0

        def stage_m3(st):
            m2, As, A2, A3, A4, w0 = st
            pM3 = emit_products(m2, A4)
            return pM3, As, A2, A3, w0

        def stage_tchain(st):
            pM3, As, A2, A3, w0 = st
            return coeff_tree(pM3, 0, w0, (As, A2, A3), "t")

        def stage_square(q, T, k):
            pT = emit_products(T, T)
            if k < S_SQUARE - 1:
                return copy_out(pT, "t2")
            # final: stage into the grouped out tile
            g = q // GRP
            if q % GRP == 0:
                og_t = sb.tile([N, GRP * QUAD * N], dt.float32, tag="og")
                og_tiles[g] = og_t
            stage = og_tiles[g]
            nc.scalar.copy(
                stage[:, (q % GRP) * QUAD * N:((q % GRP) + 1) * QUAD * N], pT[:])
            if q % GRP == GRP - 1:
                gq = q // GRP
                dst = out[GRP * QUAD * gq:GRP * QUAD * (gq + 1)].rearrange(
                    "m p c -> p m c")
                nc.sync.dma_start(
                    dst, og_tiles[g][:].rearrange("p (m c) -> p m c", c=N))
            return None

        # rolling software pipeline, GROUP_W quads wide:
        # iter i emits loads/powers/M2/C1/M3/Tchain of group i while the
        # squaring rounds of group i-1 are interspersed as PE gap fillers.
        GROUP_W = 2
        prev_grp = None
        prev_T = None
        sq_next = 0

        def emit_square_round(gq, T_list):
            """Emit HALF a squaring round (one quad) per call: finer fill grain."""
            nonlocal sq_next
            if gq is None or sq_next >= 2 * S_SQUARE:
                return T_list
            k, i = sq_next // 2, sq_next % 2
            T_list[i] = stage_square(gq[i], T_list[i], k)
            sq_next += 1
            return T_list

        for rep in range(repeat):
          for q0 in range(0, N_QUADS, GROUP_W):
            grp = list(range(q0, q0 + GROUP_W))
            As_l = [stage_load(q) for q in grp]
            st = []
            for i in range(GROUP_W):
                st.append(stage_powers(As_l[i]))
                prev_T = emit_square_round(prev_grp, prev_T)
                prev_T = emit_square_round(prev_grp, prev_T)
            st = [stage_m2(s) for s in st]
            prev_T = emit_square_round(prev_grp, prev_T)
            prev_T = emit_square_round(prev_grp, prev_T)
            st = [stage_c1chain(s) for s in st]
            prev_T = emit_square_round(prev_grp, prev_T)
            while prev_grp is not None and sq_next < 2 * S_SQUARE:
                prev_T = emit_square_round(prev_grp, prev_T)
            st = [stage_m3(s) for s in st]
            T_l = [stage_tchain(s) for s in st]
            prev_grp, prev_T, sq_next = grp, T_l, 0

        while sq_next < 2 * S_SQUARE:
            prev_T = emit_square_round(prev_grp, prev_T)

    nc.compile()
    return nc


_NC_CACHE = None


def kernel(x) -> np.ndarray:
    global _NC_CACHE
    x = np.asarray(x, dtype=np.float32)
    assert x.shape == (B_FULL, N, N), x.shape
    if _NC_CACHE is None:
        _NC_CACHE = build_bass()
    nc = _NC_CACHE

    consts = _consts_np()
    in_maps = [
        {"x": np.ascontiguousarray(x[i * B_CORE:(i + 1) * B_CORE]), "consts": consts}
        for i in range(N_CORES)
    ]
    res = run_bass_kernel_spmd(nc, in_maps, list(range(N_CORES))).results
    return np.concatenate([r["out"] for r in res], axis=0)

